# revision 59
# baseline (speedup 1.0000x reference)
"""Trainium2 Bass kernel for nn_Attention_5927054869144.

Channel-attention over [B=8, C=64, H=256, W=256] inputs. Data-parallel over
batch: one batch element per NeuronCore (8 cores), no collectives.

Per-core pipeline (x_b viewed as [64, 65536], spatial blocks of 8192):
  1. qkvT projection with x-chunk stationary on the PE -> q/k/v directly in
     spatial-partition layout (fp16 operands, fp32 PSUM).
  2. Per-head-pair dots matmuls from gathered (alpha, i) column APs,
     accumulated in PSUM over all spatial tiles (softmax scale folded into
     Wq/bq host-side).
  3. Unnormalized softmax: exp(x - max) on ScalarE with accumulated row sums;
     1/rowsum folded into per-head copies of Wo^T.
  4. M_h^T = expdots_h @ (Wo^T * recip) via tiny matmuls.
  5. v transposed to dim-partition layout via TensorE gather-transposes
     (fp16 PSUM), then final output = M_h^T.T @ v_dp, evacuated fp32 and
     DMAed per 1MB chunk round-robined across DMA queues.

Perf notes vs the original version: PE warm-up matmuls keep the HAM clock
gate at 2.4 GHz from the first projection; x is loaded in 1 MB per-(quarter,
block-pair) tiles so the first matmul starts ~4us in; deeper PSUM/slot
buffering removes PE stalls on evacuations; output is stored in 1 MB chunks
issued as computed so the store DMA overlaps phase B compute.
"""

import os
import sys

import numpy as np

for _p in ("/opt/trn_rl_repo", "/root/.axon_site/_ro/trn_rl_repo"):
    if os.path.isdir(_p) and _p not in sys.path:
        sys.path.insert(0, _p)

from concourse import bacc, mybir, tile  # noqa: E402
from concourse.bass_utils import run_bass_kernel_spmd  # noqa: E402

F32 = mybir.dt.float32
F16 = mybir.dt.float16

HEADS = 8
C = 64
HW = 65536          # 256*256 spatial positions per batch element
BL = HW // HEADS    # 8192, per-head block length
NQ = 4              # spatial quarters (within-block n ranges)
QL = BL // NQ       # 2048 within-block positions per quarter
TPQ = QL // 128     # 16 tile groups per quarter
N_GROUPS = BL // 128  # 64 total tile groups
CHUNK_B = 2048      # phase-B output chunk columns per head pair (1 MB)
N_WARM = 80         # PE warm-up matmuls (~4us) to flip HAM to 2.4 GHz

LAST_RESULTS = None


def _build_kernel(hw=HW):
    bl = hw // HEADS
    ql = bl // NQ
    tpq = ql // 128
    n_groups = bl // 128
    chunk_b = min(CHUNK_B, bl)
    s5n = chunk_b // 512
    c0n = bl // chunk_b

    nc = bacc.Bacc("TRN2", target_bir_lowering=False, debug=False)
    x_d = nc.dram_tensor("x", [65, hw], F32, kind="ExternalInput")
    wqkv_d = nc.dram_tensor("wqkv", [65, 192], F16, kind="ExternalInput")
    wot_d = nc.dram_tensor("wot", [128, 64], F32, kind="ExternalInput")
    ident_d = nc.dram_tensor("ident", [128, 128], F16, kind="ExternalInput")
    # fp16 output in evac-native layout: row s*64+o holds, for each pair pr,
    # block 2*pr+s of channel o at cols pr*bl. Host decodes + casts to fp32.
    # (Long per-partition runs -> fast store; fp32 chunked stores measured
    # 50-65 GB/s vs ~130 GB/s for this flat fp16 pattern.)
    out_d = nc.dram_tensor("out", [128, hw // 2], F16, kind="ExternalOutput")

    x_ap = x_d.ap()
    out_ap = out_d.ap()

    with tile.TileContext(nc) as tc:
        with (
            tc.tile_pool(name="consts", bufs=1) as cpool,
            tc.tile_pool(name="pers", bufs=1) as pers,
            tc.tile_pool(name="dotsp", bufs=1, space="PSUM") as dotspool,
        ):
            wqkv_sb = cpool.tile([65, 192], F16)
            wot_sb = cpool.tile([128, 64], F32)
            ident_sb = cpool.tile([128, 128], F16)
            nc.sync.dma_start(out=wqkv_sb[:, :], in_=wqkv_d.ap()[:, :])
            nc.sync.dma_start(out=wot_sb[:, :], in_=wot_d.ap()[:, :])
            nc.sync.dma_start(out=ident_sb[:, :], in_=ident_d.ap()[:, :])

            # v in dim-partition layout: [pair, d(0:64 even head / 64:128 odd), n]
            vdp = pers.tile([128, 4 * bl], F16)
            # all 4 pairs' dots share one PSUM bank. Never use start=True
            # here: a start's whole-bank has_written clear can race the
            # neighboring pairs' first drains (observed intermittent g0 loss
            # under shifted timing). Instead zero the bank once via DVE and
            # let every matmul accumulate / overwrite-on-clear-bits.
            dots_big = dotspool.tile([128, 512], F32, name="dots")
            nc.vector.memset(dots_big[:, :], 0.0)
            dots_ps = [dots_big[:, 128 * p:128 * p + 128] for p in range(4)]

            # ---------------- Phase A ----------------
            vdp_v = vdp.rearrange("p (r n) -> p r n", r=4)
            with (
                tc.tile_pool(name="xq", bufs=12) as xpool,
                tc.tile_pool(name="slots", bufs=9) as slotpool,
                tc.tile_pool(name="projp", bufs=5, space="PSUM") as projpool,
                tc.tile_pool(name="vtrp", bufs=2, space="PSUM") as vtrpool,
            ):
                # PE warm-up: ~4us of back-to-back tiny matmuls during the
                # first x DMA so HAM un-throttles before real work arrives.
                # memset-sourced weights: no dependency on any const DMA.
                warm_w = cpool.tile([65, 128], F16)
                nc.vector.memset(warm_w[:, :], 1.0)
                warm_tile = projpool.tile([128, 384], F32, name="pp")
                warm_ps = warm_tile[0:64, 0:64]
                for _ in range(96):
                    nc.tensor.matmul(
                        warm_ps[:, :],
                        lhsT=warm_w[:, 0:64],
                        rhs=warm_w[:, 64:128],
                        start=True,
                        stop=True,
                    )

                x_blk = x_ap.rearrange("p (i n) -> p i n", i=8)
                slots = {}

                def consume(g):
                    # dots + v-transpose for a group whose slot is fully evac'd
                    slot = slots.pop(g)
                    vt = vtrpool.tile([128, 512], F16, name="vt")
                    for pr in range(4):
                        qs = slot[:, 128 * pr: 128 * pr + 128]
                        ks = slot[:, 512 + 128 * pr: 512 + 128 * pr + 128]
                        vs = slot[:, 1024 + 128 * pr: 1024 + 128 * pr + 128]
                        nc.tensor.matmul(
                            dots_ps[pr][:, :],
                            lhsT=qs,
                            rhs=ks,
                            start=False,
                            stop=(g == n_groups - 1),
                        )
                        nc.tensor.transpose(
                            vt[:, pr * 128:(pr + 1) * 128], vs, ident_sb[:, :]
                        )
                    voff = g * 128
                    vdst = vdp_v[:, :, voff:voff + 128]
                    # vector does 16-bit copies ~1.6x faster than scalar;
                    # give it 2 of every 3
                    if g % 3 != 2:
                        nc.vector.tensor_copy(vdst, vt[:, :])
                    else:
                        nc.scalar.copy(vdst, vt[:, :])

                for q in range(NQ):
                    # per-(quarter, block-pair) x tiles: 1 MB DMAs, so the
                    # first projection starts ~4us in and later tiles
                    # prefetch behind compute on the gpsimd SWDGE queue.
                    xqt = []
                    for ip in range(4):
                        xq = xpool.tile([65, 2 * ql], F16, name="xq")
                        # two half-DMAs per tile: SWDGE descriptor-gen for
                        # the first half overlaps SDMA execution sooner
                        xq_v = xq.rearrange("p (c n) -> p c n", c=2)
                        nh = 4 if q == 0 else 2
                        hl = ql // nh
                        for hh in range(nh):
                            nc.gpsimd.dma_start(
                                out=xq_v[:, :, hh * hl:(hh + 1) * hl],
                                in_=x_blk[:, 2 * ip:2 * ip + 2,
                                          q * ql + hh * hl:
                                          q * ql + (hh + 1) * hl],
                            )
                        xqt.append(xq)
                    for t0 in range(tpq):
                        g = q * tpq + t0
                        # slot cols: r*512 + head*64 + i*8 + alpha (alpha contiguous)
                        slot = slotpool.tile([128, 1536], F16, name="slot")
                        slot_sc = slot.rearrange(
                            "p (r h i a) -> p i r h a", r=3, h=8, i=8, a=8
                        )
                        slots[g] = slot
                        for ip in range(4):  # chunk pairs (2i, 2i+1)
                            pp = projpool.tile([128, 384], F32, name="pp")
                            for c in range(2):
                                nc.tensor.matmul(
                                    pp[:, c * 192:(c + 1) * 192],
                                    lhsT=xqt[ip][:, c * ql + t0 * 128:
                                                  c * ql + t0 * 128 + 128],
                                    rhs=wqkv_sb[:, :],
                                    start=True,
                                    stop=True,
                                )
                            dst = slot_sc[:, 2 * ip: 2 * ip + 2, :, :, :]
                            if ip % 2 == 0:
                                nc.vector.tensor_copy(dst, pp[:, :])
                            else:
                                nc.scalar.copy(dst, pp[:, :])
                        if g >= 3:
                            consume(g - 3)
                for g in (n_groups - 3, n_groups - 2, n_groups - 1):
                    consume(g)

            # ---------------- Softmax + output ----------------
            with (
                tc.tile_pool(name="smx", bufs=1) as smx,
                tc.tile_pool(name="mhp", bufs=2, space="PSUM") as mhpool,
                tc.tile_pool(name="finp", bufs=5, space="PSUM") as finpool,
                tc.tile_pool(name="outs", bufs=1) as outpool,
            ):
                # bridge the softmax DVE latency with PE warm-up matmuls so
                # HAM doesn't re-throttle before the output pass
                wbr = finpool.tile([128, 512], F32, name="fp_")
                for _ in range(48):
                    nc.tensor.matmul(
                        wbr[0:64, 0:64],
                        lhsT=wqkv_sb[:, 0:64],
                        rhs=wqkv_sb[:, 64:128],
                        start=True,
                        stop=True,
                    )

                negmax = smx.tile([128, 4], F32)
                rowsum = smx.tile([128, 4], F32)
                recip = smx.tile([128, 4], F32)
                exps = smx.tile([128, 4 * 64], F16)
                wots = smx.tile([128, 4 * 64], F16)
                mh_sb = smx.tile([128, 4 * 64], F16)
                def softmax_head(h):
                    b = (h % 2) * 64
                    pr = h // 2
                    dsl = dots_ps[pr][b:b + 64, b:b + 64]
                    nc.vector.reduce_max(
                        negmax[b:b + 64, pr:pr + 1], dsl,
                        axis=mybir.AxisListType.X, negate=True,
                    )
                    nc.scalar.activation(
                        exps[b:b + 64, pr * 64:(pr + 1) * 64], dsl,
                        mybir.ActivationFunctionType.Exp,
                        bias=negmax[b:b + 64, pr:pr + 1],
                        scale=1.0,
                        accum_out=rowsum[b:b + 64, pr:pr + 1],
                    )
                    nc.vector.reciprocal(
                        recip[b:b + 64, pr:pr + 1], rowsum[b:b + 64, pr:pr + 1]
                    )
                    nc.vector.tensor_scalar_mul(
                        wots[b:b + 64, pr * 64:(pr + 1) * 64],
                        wot_sb[b:b + 64, :],
                        recip[b:b + 64, pr:pr + 1],
                    )

                def mh_pair(pr):
                    # per-pair PSUM tile (pool bufs=2) so the next pair's mh
                    # can be computed while the current pair's fins run
                    ps = mhpool.tile([128, 64], F32, name="mh_ps")
                    for s in range(2):
                        b = s * 64
                        nc.tensor.matmul(
                            ps[b:b + 64, :],
                            lhsT=exps[b:b + 64, pr * 64:(pr + 1) * 64],
                            rhs=wots[b:b + 64, pr * 64:(pr + 1) * 64],
                            start=True,
                            stop=True,
                        )
                        nc.vector.tensor_copy(
                            mh_sb[b:b + 64, pr * 64:(pr + 1) * 64],
                            ps[b:b + 64, :],
                        )

                # out accumulates in four flat fp16 tiles (row = s*64+o,
                # col = n), each stored right after its pair's evacs finish
                # so the store overlaps the next pair's compute.
                quarts = [outpool.tile([128, bl], F16, name=f"oq{i}")
                          for i in range(4)]
                st_engs = [nc.sync, nc.scalar, nc.gpsimd, nc.sync]
                # all DVE/ACT softmax chains run up front, hidden under the
                # PE warm bridge + first pair's output matmuls
                for h in range(HEADS):
                    softmax_head(h)
                mh_pair(0)
                for pr in range(4):
                    half = quarts[pr]
                    cbase = 0
                    for t in range(bl // 512):
                        fp_ = finpool.tile([128, 512], F32, name="fp_")
                        n0 = pr * bl + t * 512
                        nc.tensor.matmul(
                            fp_[0:64, :],
                            lhsT=mh_sb[0:64, pr * 64:(pr + 1) * 64],
                            rhs=vdp[0:64, n0:n0 + 512],
                            start=True,
                            stop=True,
                        )
                        nc.tensor.matmul(
                            fp_[64:128, :],
                            lhsT=mh_sb[64:128, pr * 64:(pr + 1) * 64],
                            rhs=vdp[64:128, n0:n0 + 512],
                            start=True,
                            stop=True,
                        )
                        dst = half[:, cbase + t * 512:cbase + (t + 1) * 512]
                        if t % 2 == 0:
                            nc.vector.tensor_copy(dst, fp_[:, :])
                        else:
                            nc.scalar.copy(dst, fp_[:, :])
                        if t == 11 and pr < 3:
                            # issue the next pair's mh early so its copy
                            # clears the vector queue before fins need it
                            mh_pair(pr + 1)
                    st_engs[pr].dma_start(
                        out=out_ap[:, pr * bl:(pr + 1) * bl],
                        in_=half[:, :])

    nc.compile()
    return nc


_NC_CACHE = {}


def _get_nc(hw=HW):
    if hw not in _NC_CACHE:
        _NC_CACHE[hw] = _build_kernel(hw)
    return _NC_CACHE[hw]


def _host_inputs(Wq, bq, Wk, bk, Wv, bv, Wo):
    scale = 64 ** -0.5
    wqkv = np.zeros((65, 192), np.float16)
    wqkv[:64, 0:64] = (Wq.T * scale).astype(np.float16)
    wqkv[64, 0:64] = (bq * scale).astype(np.float16)
    wqkv[:64, 64:128] = Wk.T.astype(np.float16)
    wqkv[64, 64:128] = bk.astype(np.float16)
    wqkv[:64, 128:192] = Wv.T.astype(np.float16)
    wqkv[64, 128:192] = bv.astype(np.float16)
    # kernel uses c' = i*8 + alpha ordering; original c = alpha*8 + i
    pi = np.array([(c % 8) * 8 + c // 8 for c in range(64)])
    wotp = Wo.T[pi]
    wot = np.concatenate([wotp, wotp], axis=0).astype(np.float32)
    ident = np.eye(128, dtype=np.float16)
    return wqkv, wot, ident


def kernel(x, Wq, bq, Wk, bk, Wv, bv, Wo):
    global LAST_RESULTS
    B = x.shape[0]
    hw = x.shape[2] * x.shape[3]
    nc = _get_nc(hw)
    wqkv, wot, ident = _host_inputs(Wq, bq, Wk, bk, Wv, bv, Wo)

    in_maps = []
    for bidx in range(B):
        x65 = np.empty((65, hw), np.float32)
        x65[:64] = x[bidx].reshape(64, hw)
        x65[64] = 1.0
        in_maps.append({"x": x65, "wqkv": wqkv, "wot": wot, "ident": ident})

    trace = bool(os.environ.get("KERNEL_TRACE"))
    res = run_bass_kernel_spmd(
        nc, in_maps, core_ids=list(range(B)), trace=trace
    )
    LAST_RESULTS = res
    bl = hw // HEADS
    # decode [128, hw/2] fp16 rows s*64+o, cols pr*bl+n -> [64, 8, bl] fp32
    out = np.stack(
        [res.results[bidx]["out"].reshape(2, 64, 4, bl)
         .transpose(1, 2, 0, 3).reshape(64, HEADS, bl).astype(np.float32)
         for bidx in range(B)]
    )
    return out
